# revision 6
# baseline (speedup 1.0000x reference)
"""Trainium2 Bass kernel for nn_Linear_18494129177115 (moe_routing).

Math (reference, fp32):
  base   = x @ W^T                                  [B,T,O]
  logits = x @ Wr^T + lang_bias                     [B,T,E]
  gates  = scatter(softmax(top2(logits)))           [B,T,E]
  h      = x @ A_e^T  (all experts)                 [B,T,E,R]
  out    = base + SCALING * sum_e gates_e * h_e @ B_e^T

Key design points:
- With A_cat = concat_e(A_e) [E*R, D] and B_cat[e*R+r, o] = B[e, o, r],
  the gated LoRA collapses to
    out = x @ W^T + (gates_expanded * (x @ A_cat^T)) @ (SCALING * B_cat),
  two thin matmuls fused into the base GEMM's PSUM accumulation.
- The tolerance gate is 2e-2 absmax-relative; a SINGLE bf16 pass of the
  heavy GEMMs lands at ~2.3e-3 (measured vs fp32 reference on the
  grading seed), so the base GEMM, h, and the B_cat matmul all run one
  bf16 pass (3x fewer PE cycles than the hi/lo 3-pass split).
- The router alone keeps a 3-pass bf16 hi/lo split (xh@Wrh + xl@Wrh +
  xh@Wrl): top-2 selection flips from single-pass logit noise cost up
  to ~1e-2 absmax; 3-pass logits are ~1e-5 accurate so selection
  matches fp32. Wr is tiny so this costs ~30us of PE.
- DMA: all host-side layouts are partition-major so every transfer is
  a contiguous >=1MB block; W streams in 1MB chunks over the two HWDGE
  rings, x-lo streams on the SWDGE ring, outputs pair-batched to 512KB.

Sharding: data-parallel over tokens, 1024 tokens/core on 8 cores; all
weights replicated; no collectives. Each core's tokens lie in a single
batch row, so the language bias is a per-core constant [E,1] column.
"""

import numpy as np

LANG_BIAS = 5.0
SCALING = 32.0 / 16.0
B_SZ, T_SZ, D_SZ, O_SZ, E_SZ, R_SZ = 4, 2048, 4096, 4096, 8, 16
NCORES = 8
TPC = (B_SZ * T_SZ) // NCORES      # 1024 tokens per core
NT = TPC // 128                    # 8 token tiles per core
NK = D_SZ // 128                   # 32 contraction chunks
NO = O_SZ // 512                   # 8 output tiles of 512
ER = E_SZ * R_SZ                   # 128 (expert, rank) pairs
NEG_BIG = -(2.0 ** 100)

_CACHE: dict = {}
LAST_RESULT = None


def _build_bass():
    import concourse.bacc as bacc
    import concourse.mybir as mybir
    from concourse import tile
    from concourse.masks import make_identity

    f32 = mybir.dt.float32
    bf16 = mybir.dt.bfloat16
    AX = mybir.AxisListType.X
    OP = mybir.AluOpType
    ACT = mybir.ActivationFunctionType

    nc = bacc.Bacc(None, target_bir_lowering=False, debug=False)

    # x hi, partition-major: [128, kc, t]
    xh_d = nc.dram_tensor("xh", [128, NK, TPC], bf16, kind="ExternalInput")
    # x lo, grouped for streaming: [g4, 128, 4, t]
    xl_d = nc.dram_tensor("xl", [NK // 4, 128, 4, TPC], bf16, kind="ExternalInput")
    # W^T stream: per (ot, g8) a [128, 8kc, 512] block
    wt_d = nc.dram_tensor("wt", [NO, NK // 8, 128, 8, 512], bf16, kind="ExternalInput")
    # A_cat^T resident: [128, kc, ER]
    acat_d = nc.dram_tensor("acat", [128, NK, ER], bf16, kind="ExternalInput")
    # Wr^T hi/lo: [2, 128, kc, E]
    wrt_d = nc.dram_tensor("wrt", [2, 128, NK, E_SZ], bf16, kind="ExternalInput")
    # SCALING * B_cat, bf16: [ER, O]
    bcat_d = nc.dram_tensor("bcat", [ER, O_SZ], bf16, kind="ExternalInput")
    # language bias column [E, 1]
    bias_d = nc.dram_tensor("biasr", [E_SZ, 1], f32, kind="ExternalInput")
    # expert -> (expert, rank) one-hot expansion [E, ER]
    sel_d = nc.dram_tensor("sel", [E_SZ, ER], f32, kind="ExternalInput")
    out_d = nc.dram_tensor("out", [NO, NT // 2, 128, 2, 512], f32, kind="ExternalOutput")

    with tile.TileContext(nc) as tc:
        with (
            tc.tile_pool(name="const", bufs=1) as cpool,
            tc.tile_pool(name="wstream", bufs=3) as wpool,
            tc.tile_pool(name="xlstream", bufs=2) as xlpool,
            tc.tile_pool(name="ostage", bufs=3) as opool,
            tc.tile_pool(name="gate", bufs=2) as gpool,
            tc.tile_pool(name="psum", bufs=8, space="PSUM") as psum,
        ):
            # ---- resident inputs ----
            xh_g = [
                cpool.tile([128, 8, TPC], bf16, name=f"xh_g{g}") for g in range(4)
            ]
            acat_sb = cpool.tile([128, NK, ER], bf16, name="acat_sb")
            wrh_sb = cpool.tile([128, NK, E_SZ], bf16, name="wrh_sb")
            wrl_sb = cpool.tile([128, NK, E_SZ], bf16, name="wrl_sb")
            bch_sb = cpool.tile([ER, O_SZ], bf16, name="bch_sb")
            bias_sb = cpool.tile([E_SZ, 1], f32, name="bias_sb")
            sel_sb = cpool.tile([E_SZ, ER], f32, name="sel_sb")
            ident_sb = cpool.tile([128, 128], f32, name="ident_sb")
            hT_sb = cpool.tile([128, TPC], f32, name="hT_sb")
            ghh_sb = cpool.tile([128, NT, 128], bf16, name="ghh_sb")
            lgT_sb = cpool.tile([E_SZ, TPC], f32, name="lgT_sb")

            def xh(kc):
                return xh_g[kc // 8][:, kc % 8, :]

            # tiny weights first so phase 1 isn't gated on the big x DMAs
            nc.sync.dma_start(wrh_sb[:], wrt_d[0])
            nc.sync.dma_start(wrl_sb[:], wrt_d[1])
            nc.scalar.dma_start(bias_sb[:], bias_d[:])
            nc.scalar.dma_start(sel_sb[:], sel_d[:])
            for g in range(4):
                ksl = slice(g * 8, (g + 1) * 8)
                eng = nc.sync if g < 2 else nc.scalar
                eng.dma_start(xh_g[g][:], xh_d[:, ksl, :])
            nc.sync.dma_start(acat_sb[:], acat_d[:])
            nc.scalar.dma_start(bch_sb[:], bcat_d[:])
            make_identity(nc, ident_sb[:])

            # ---- phase 1a: router logits (3-pass bf16 hi/lo) ----
            plT = [
                psum.tile([E_SZ, 512], f32, tag="bank", name=f"plT{t}")
                for t in range(TPC // 512)
            ]
            for kg in range(NK // 4):
                xl_t = xlpool.tile([128, 4, TPC], bf16, name="xl_t")
                nc.gpsimd.dma_start(xl_t[:], xl_d[kg])
                for k4 in range(4):
                    kc = kg * 4 + k4
                    first = kc == 0
                    last = kc == NK - 1
                    for tb in range(TPC // 512):
                        sl = slice(tb * 512, (tb + 1) * 512)
                        nc.tensor.matmul(
                            plT[tb][:], wrh_sb[:, kc, :], xh(kc)[:, sl],
                            start=first, stop=False,
                        )
                        nc.tensor.matmul(
                            plT[tb][:], wrh_sb[:, kc, :], xl_t[:, k4, sl],
                            start=False, stop=False,
                        )
                        nc.tensor.matmul(
                            plT[tb][:], wrl_sb[:, kc, :], xh(kc)[:, sl],
                            start=False, stop=last,
                        )
            for tb in range(TPC // 512):
                sl = slice(tb * 512, (tb + 1) * 512)
                # fold the language bias into the PSUM drain (bias is a
                # per-partition [E,1] column in this transposed layout)
                nc.vector.tensor_scalar(
                    lgT_sb[:, sl], plT[tb][:], bias_sb[:], None, op0=OP.add
                )

            # ---- phase 1b: h = A_cat @ x^T, with the per-token-tile gate
            # softmax chain interleaved so its DVE work hides under the MMs
            ph = [
                psum.tile([128, 512], f32, tag="bank", name=f"ph{t}")
                for t in range(TPC // 512)
            ]
            gates_t = []

            def gate_chain(tt):
                ts = slice(tt * 128, (tt + 1) * 128)
                plg = psum.tile([128, E_SZ], f32, tag="bank", name=f"plg{tt}")
                nc.tensor.transpose(plg[:], lgT_sb[:, ts], ident_sb[:E_SZ, :E_SZ])
                m1 = gpool.tile([128, 1], f32, name="m1")
                nc.vector.reduce_max(m1[:], plg[:], axis=AX)
                mask1 = gpool.tile([128, E_SZ], f32, name="mask1")
                nc.vector.tensor_scalar(
                    mask1[:], plg[:], m1[:], None, op0=OP.is_equal
                )
                l2 = gpool.tile([128, E_SZ], f32, name="l2")
                nc.vector.tensor_scalar(l2[:], mask1[:], NEG_BIG, None, op0=OP.mult)
                nc.vector.tensor_tensor(l2[:], l2[:], plg[:], op=OP.add)
                m2 = gpool.tile([128, 1], f32, name="m2")
                nc.vector.reduce_max(m2[:], l2[:], axis=AX)
                mask2 = gpool.tile([128, E_SZ], f32, name="mask2")
                nc.vector.tensor_scalar(
                    mask2[:], l2[:], m2[:], None, op0=OP.is_equal
                )
                w1 = gpool.tile([128, 1], f32, name="w1")
                nc.scalar.activation(
                    w1[:], m2[:], ACT.Sigmoid, bias=m1[:], scale=-1.0
                )
                w2 = gpool.tile([128, 1], f32, name="w2")
                nc.vector.tensor_scalar(
                    w2[:], w1[:], -1.0, 1.0, op0=OP.mult, op1=OP.add
                )
                g1 = gpool.tile([128, E_SZ], f32, name="g1")
                nc.vector.tensor_scalar(g1[:], mask1[:], w1[:], None, op0=OP.mult)
                gates = gpool.tile([128, E_SZ], f32, name=f"gates{tt}")
                nc.vector.tensor_scalar(
                    gates[:], mask2[:], w2[:], None, op0=OP.mult
                )
                nc.vector.tensor_tensor(gates[:], gates[:], g1[:], op=OP.add)
                gates_t.append(gates)

            for kc in range(NK):
                first = kc == 0
                last = kc == NK - 1
                for tb in range(TPC // 512):
                    sl = slice(tb * 512, (tb + 1) * 512)
                    nc.tensor.matmul(
                        ph[tb][:], acat_sb[:, kc, :], xh(kc)[:, sl],
                        start=first, stop=last,
                    )
                if kc % 4 == 3:
                    gate_chain(kc // 4)
            for tb in range(TPC // 512):
                sl = slice(tb * 512, (tb + 1) * 512)
                nc.vector.tensor_copy(hT_sb[:, sl], ph[tb][:])

            # ---- phase 1c: expand gates to (e,r) rows, ghh = gates_exp * h
            for tt in range(NT):
                ts = slice(tt * 128, (tt + 1) * 128)
                ptr = psum.tile([E_SZ, 128], f32, tag="bank", name=f"ptr{tt}")
                nc.tensor.transpose(ptr[:], gates_t[tt][:], ident_sb[:])
                gT = gpool.tile([E_SZ, 128], f32, name="gT")
                nc.vector.tensor_copy(gT[:], ptr[:])
                pge = psum.tile([128, 128], f32, tag="bank", name=f"pge{tt}")
                nc.tensor.matmul(pge[:], sel_sb[:], gT[:], start=True, stop=True)
                nc.vector.tensor_tensor(
                    ghh_sb[:, tt, :], pge[:], hT_sb[:, ts], op=OP.mult
                )

            # ---- phase 2: out = x @ W^T (+ ghh @ SCALING*B_cat) ----
            for ot in range(NO):
                po = [
                    psum.tile([128, 512], f32, tag="bank", name=f"po{ot}_{i}")
                    for i in range(NT)
                ]
                for g in range(NK // 8):
                    w_t = wpool.tile([128, 8, 512], bf16, name="w_t")
                    eng = nc.sync if (ot * 4 + g) % 2 == 0 else nc.scalar
                    eng.dma_start(w_t[:], wt_d[ot, g])
                    for k8 in range(8):
                        kc = g * 8 + k8
                        for tt in range(NT):
                            nc.tensor.matmul(
                                po[tt][:],
                                xh(kc)[:, tt * 128 : (tt + 1) * 128],
                                w_t[:, k8, :],
                                start=(kc == 0),
                                stop=False,
                            )
                osl = slice(ot * 512, (ot + 1) * 512)
                for pair in range(NT // 2):
                    ob = opool.tile([128, 2, 512], f32, name="ob")
                    for j in range(2):
                        tt = pair * 2 + j
                        nc.tensor.matmul(
                            po[tt][:], ghh_sb[:, tt, :], bch_sb[:, osl],
                            start=False, stop=True,
                        )
                        nc.vector.tensor_copy(ob[:, j, :], po[tt][:])
                    oeng = (nc.gpsimd, nc.sync, nc.scalar)[(ot * 4 + pair) % 3]
                    oeng.dma_start(out_d[ot, pair], ob[:])

    nc.compile()
    return nc


def _split_bf16(a):
    import ml_dtypes

    hi = a.astype(ml_dtypes.bfloat16)
    lo = (a - hi.astype(np.float32)).astype(ml_dtypes.bfloat16)
    return hi, lo


def _host_prep(x, language_ids, W, Wr, A, B):
    x = np.asarray(x, dtype=np.float32)
    W = np.asarray(W, dtype=np.float32)
    Wr = np.asarray(Wr, dtype=np.float32)
    A = np.asarray(A, dtype=np.float32)
    B = np.asarray(B, dtype=np.float32)
    lang = np.asarray(language_ids).astype(np.int64)

    xf = np.ascontiguousarray(x.reshape(B_SZ * T_SZ, D_SZ))

    # W^T [D,O] bf16 hi: [ot, g8, p, k8, n]
    wtT = W.T.reshape(NK, 128, NO, 512)                   # [kc, p, ot, n]
    wh, _ = _split_bf16(wtT)
    wt = np.ascontiguousarray(
        wh.reshape(NK // 8, 8, 128, NO, 512).transpose(3, 0, 2, 1, 4)
    )

    acat_t = np.ascontiguousarray(A.reshape(ER, D_SZ).T).reshape(NK, 128, ER)
    ah, _ = _split_bf16(acat_t)
    acat = np.ascontiguousarray(ah.transpose(1, 0, 2))    # [p, kc, ER]

    wrtT = np.ascontiguousarray(Wr.T).reshape(NK, 128, E_SZ)
    wrh, wrl = _split_bf16(wrtT)
    wrt = np.ascontiguousarray(
        np.stack([wrh, wrl], axis=0).transpose(0, 2, 1, 3)  # [2, p, kc, E]
    )

    bcat32 = (SCALING * B.transpose(0, 2, 1)).reshape(ER, O_SZ)
    bh, _ = _split_bf16(bcat32)
    bcat = np.ascontiguousarray(bh)

    sel = np.zeros((E_SZ, ER), dtype=np.float32)
    sel[np.arange(ER) // R_SZ, np.arange(ER)] = 1.0

    in_maps = []
    for c in range(NCORES):
        shard = xf[c * TPC : (c + 1) * TPC]
        xr = np.ascontiguousarray(shard.T).reshape(NK, 128, TPC)
        xhh, xll = _split_bf16(xr)
        xh = np.ascontiguousarray(xhh.transpose(1, 0, 2))             # [p, kc, t]
        xl = np.ascontiguousarray(
            xll.reshape(NK // 4, 4, 128, TPC).transpose(0, 2, 1, 3)   # [g, p, 4, t]
        )
        b = int(lang[(c * TPC) // T_SZ])
        brow = np.zeros((E_SZ, 1), dtype=np.float32)
        if b >= 0:
            brow[b, 0] = LANG_BIAS
        in_maps.append(
            {
                "xh": xh,
                "xl": xl,
                "wt": wt,
                "acat": acat,
                "wrt": wrt,
                "bcat": bcat,
                "biasr": brow,
                "sel": sel,
            }
        )
    return in_maps


def kernel(x, language_ids, W, Wr, A, B):
    global LAST_RESULT
    from concourse.bass_utils import run_bass_kernel_spmd

    if "nc" not in _CACHE:
        _CACHE["nc"] = _build_bass()
    nc = _CACHE["nc"]

    in_maps = _host_prep(x, language_ids, W, Wr, A, B)
    res = run_bass_kernel_spmd(nc, in_maps, core_ids=list(range(NCORES)))
    LAST_RESULT = res
    outs = [
        r["out"].transpose(1, 3, 2, 0, 4).reshape(TPC, O_SZ) for r in res.results
    ]
    return np.concatenate(outs, axis=0).reshape(B_SZ, T_SZ, O_SZ)


# revision 15
# speedup vs baseline: 949.0154x; 949.0154x over previous
"""Trainium2 Bass kernel for nn_Linear_18494129177115 (moe_routing).

Math (reference, fp32):
  base   = x @ W^T                                  [B,T,O]
  logits = x @ Wr^T + lang_bias                     [B,T,E]
  gates  = scatter(softmax(top2(logits)))           [B,T,E]
  h      = x @ A_e^T  (all experts)                 [B,T,E,R]
  out    = base + SCALING * sum_e gates_e * h_e @ B_e^T

Key design points:
- With A_cat = concat_e(A_e) [E*R, D] and B_cat[e*R+r, o] = B[e, o, r],
  the gated LoRA collapses to
    out = x @ W^T + (gates_expanded * (x @ A_cat^T)) @ (SCALING * B_cat),
  two thin matmuls fused into the base GEMM's PSUM accumulation.
- The tolerance gate is 2e-2 absmax-relative; a SINGLE bf16 pass of the
  heavy GEMMs lands at ~2.3e-3 (measured vs fp32 reference on the
  grading seed), so the base GEMM, h, and the B_cat matmul all run one
  bf16 pass (3x fewer PE cycles than the hi/lo 3-pass split).
- The router alone keeps a 3-pass bf16 hi/lo split (xh@Wrh + xl@Wrh +
  xh@Wrl): top-2 selection flips from single-pass logit noise cost up
  to ~1e-2 absmax; 3-pass logits are ~1e-5 accurate so selection
  matches fp32. Wr is tiny so this costs ~30us of PE.
- DMA: all host-side layouts are partition-major so every transfer is
  a contiguous >=1MB block; W streams in 1MB chunks over the two HWDGE
  rings, x-lo streams on the SWDGE ring, outputs pair-batched to 512KB.

Sharding: data-parallel over tokens, 1024 tokens/core on 8 cores; all
weights replicated; no collectives. Each core's tokens lie in a single
batch row, so the language bias is a per-core constant [E,1] column.
"""

import numpy as np

LANG_BIAS = 5.0
SCALING = 32.0 / 16.0
B_SZ, T_SZ, D_SZ, O_SZ, E_SZ, R_SZ = 4, 2048, 4096, 4096, 8, 16
NCORES = 8
TPC = (B_SZ * T_SZ) // NCORES      # 1024 tokens per core
NT = TPC // 128                    # 8 token tiles per core
NK = D_SZ // 128                   # 32 contraction chunks
NO = O_SZ // 512                   # 8 output tiles of 512
ER = E_SZ * R_SZ                   # 128 (expert, rank) pairs
NEG_BIG = -(2.0 ** 100)

_CACHE: dict = {}
LAST_RESULT = None


def _build_bass(loop_n=None):
    import concourse.bacc as bacc
    import concourse.mybir as mybir
    from concourse import tile
    from concourse.masks import make_identity

    f32 = mybir.dt.float32
    bf16 = mybir.dt.bfloat16
    AX = mybir.AxisListType.X
    OP = mybir.AluOpType
    ACT = mybir.ActivationFunctionType

    nc = bacc.Bacc(None, target_bir_lowering=False, debug=False)

    # x hi, partition-major: [128, kc, t]
    xh_d = nc.dram_tensor("xh", [128, NK, TPC], bf16, kind="ExternalInput")
    # x lo, grouped for streaming: [g4, 128, 4, t]
    xl_d = nc.dram_tensor("xl", [NK // 4, 128, 4, TPC], bf16, kind="ExternalInput")
    # W^T stream: per (ot, g8) a [128, 8kc, 512] block
    wt_d = nc.dram_tensor("wt", [NO, NK // 8, 128, 8, 512], bf16, kind="ExternalInput")
    # A_cat^T resident: [128, kc, ER]
    acat_d = nc.dram_tensor("acat", [128, NK, ER], bf16, kind="ExternalInput")
    # Wr^T hi/lo: [2, 128, kc, E]
    wrt_d = nc.dram_tensor("wrt", [2, 128, NK, E_SZ], bf16, kind="ExternalInput")
    # SCALING * B_cat, bf16: [ER, O]
    bcat_d = nc.dram_tensor("bcat", [ER, O_SZ], bf16, kind="ExternalInput")
    # language bias column [E, 1]
    bias_d = nc.dram_tensor("biasr", [E_SZ, 1], f32, kind="ExternalInput")
    # expert -> (expert, rank) one-hot expansion [E, ER]
    sel_d = nc.dram_tensor("sel", [E_SZ, ER], f32, kind="ExternalInput")
    out_d = nc.dram_tensor("out", [NO, NT // 2, 128, 2, 512], f32, kind="ExternalOutput")

    with tile.TileContext(nc) as tc:
        with (
            tc.tile_pool(name="const", bufs=1) as cpool,
            tc.tile_pool(name="wstream", bufs=3) as wpool,
            tc.tile_pool(name="xlstream", bufs=6) as xlpool,
            tc.tile_pool(name="ostage", bufs=3) as opool,
            tc.tile_pool(name="gate", bufs=2) as gpool,
            tc.tile_pool(name="psum", bufs=8, space="PSUM") as psum,
        ):

          def body(_iv=None):
            # ---- resident inputs ----
            xh_g = [
                cpool.tile([128, 4, TPC], bf16, name=f"xh_g{g}") for g in range(8)
            ]
            acat_c = [
                cpool.tile([128, 8, ER], bf16, name=f"acat_c{c}") for c in range(4)
            ]
            wrh_sb = cpool.tile([128, NK, E_SZ], bf16, name="wrh_sb")
            wrl_sb = cpool.tile([128, NK, E_SZ], bf16, name="wrl_sb")
            bch_sb = cpool.tile([ER, O_SZ], bf16, name="bch_sb")
            bias_sb = cpool.tile([E_SZ, 1], f32, name="bias_sb")
            sel_sb = cpool.tile([E_SZ, ER], f32, name="sel_sb")
            ident_sb = cpool.tile([128, 128], f32, name="ident_sb")
            hT_sb = cpool.tile([128, TPC], f32, name="hT_sb")
            ghh_sb = cpool.tile([128, NT, 128], bf16, name="ghh_sb")
            lgT_sb = cpool.tile([E_SZ, TPC], f32, name="lgT_sb")

            def xh(kc):
                return xh_g[kc // 4][:, kc % 4, :]

            # phase-1 inputs all on the sync HWDGE ring in exact demand
            # order (FIFO => no bandwidth stealing by later-needed data);
            # the W stream + bch ride the scalar ring, paced by wpool slot
            # backpressure; outputs go out on the SWDGE ring
            nc.sync.dma_start(wrh_sb[:], wrt_d[0])
            nc.sync.dma_start(wrl_sb[:], wrt_d[1])
            nc.sync.dma_start(bias_sb[:], bias_d[:])
            for g in range(8):
                ksl = slice(g * 4, (g + 1) * 4)
                nc.sync.dma_start(xh_g[g][:], xh_d[:, ksl, :])
                if g % 2 == 0:
                    c = g // 2
                    nc.sync.dma_start(acat_c[c][:], acat_d[:, c * 8 : c * 8 + 8, :])
                if g == 3:
                    nc.sync.dma_start(sel_sb[:], sel_d[:])
            xl_t = []
            for kg in range(NK // 4):
                t = xlpool.tile([128, 4, TPC], bf16, tag="xl", name=f"xl{kg}")
                nc.sync.dma_start(t[:], xl_d[kg])
                xl_t.append(t)
            nc.scalar.dma_start(bch_sb[:], bcat_d[:])
            make_identity(nc, ident_sb[:])

            # ---- phase 1: router hi-passes + h, one stream over x-hi ----
            plT = [
                psum.tile([E_SZ, 512], f32, tag="bank", name=f"plT{t}")
                for t in range(TPC // 512)
            ]
            ph = [
                psum.tile([128, 512], f32, tag="bank", name=f"ph{t}")
                for t in range(TPC // 512)
            ]
            # token-major hi-logit accumulator, one PSUM bank for all 8
            # token tiles ([128, NT*E] = 256B/partition)
            plgh = psum.tile([128, NT * E_SZ], f32, tag="bank", name="plgh")
            for kc in range(NK):
                first = kc == 0
                last = kc == NK - 1
                ac = acat_c[kc // 8][:, kc % 8, :]
                for tb in range(TPC // 512):
                    sl = slice(tb * 512, (tb + 1) * 512)
                    nc.tensor.matmul(
                        ph[tb][:], ac, xh(kc)[:, sl],
                        start=first, stop=last,
                    )
                for tt in range(NT):
                    xs = xh(kc)[:, tt * 128 : (tt + 1) * 128]
                    osl8 = slice(tt * E_SZ, (tt + 1) * E_SZ)
                    nc.tensor.matmul(
                        plgh[:, osl8], xs, wrh_sb[:, kc, :],
                        start=first, stop=False,
                    )
                    nc.tensor.matmul(
                        plgh[:, osl8], xs, wrl_sb[:, kc, :],
                        start=False, stop=last,
                    )
            # router lo-pass (xl arrived during the loop above)
            for kc in range(NK):
                last = kc == NK - 1
                for tb in range(TPC // 512):
                    sl = slice(tb * 512, (tb + 1) * 512)
                    nc.tensor.matmul(
                        plT[tb][:], wrh_sb[:, kc, :],
                        xl_t[kc // 4][:, kc % 4, sl],
                        start=False, stop=last,
                    )
            for tb in range(TPC // 512):
                sl = slice(tb * 512, (tb + 1) * 512)
                # fold the language bias into the PSUM drain (bias is a
                # per-partition [E,1] column in this transposed layout)
                nc.vector.tensor_scalar(
                    lgT_sb[:, sl], plT[tb][:], bias_sb[:], None, op0=OP.add
                )
                nc.vector.tensor_copy(hT_sb[:, sl], ph[tb][:])

            # ---- gate softmax chain (emitted inside ot=0's first half
            # below so the DVE work hides under base-GEMM matmuls)
            gates_t = [None] * NT

            def gate_chain(tt):
                ts = slice(tt * 128, (tt + 1) * 128)
                plo = psum.tile([128, E_SZ], f32, tag="bank", name=f"plo{tt}")
                nc.tensor.transpose(plo[:], lgT_sb[:, ts], ident_sb[:E_SZ, :E_SZ])
                plg = gpool.tile([128, E_SZ], f32, name="logit")
                nc.vector.tensor_tensor(
                    plg[:], plgh[:, tt * E_SZ : (tt + 1) * E_SZ], plo[:], op=OP.add
                )
                m1 = gpool.tile([128, 1], f32, name="m1")
                nc.vector.reduce_max(m1[:], plg[:], axis=AX)
                mask1 = gpool.tile([128, E_SZ], f32, name="mask1")
                nc.vector.tensor_scalar(
                    mask1[:], plg[:], m1[:], None, op0=OP.is_equal
                )
                l2 = gpool.tile([128, E_SZ], f32, name="l2")
                nc.vector.tensor_scalar(l2[:], mask1[:], NEG_BIG, None, op0=OP.mult)
                nc.vector.tensor_tensor(l2[:], l2[:], plg[:], op=OP.add)
                m2 = gpool.tile([128, 1], f32, name="m2")
                nc.vector.reduce_max(m2[:], l2[:], axis=AX)
                mask2 = gpool.tile([128, E_SZ], f32, name="mask2")
                nc.vector.tensor_scalar(
                    mask2[:], l2[:], m2[:], None, op0=OP.is_equal
                )
                w1 = gpool.tile([128, 1], f32, name="w1")
                nc.scalar.activation(
                    w1[:], m2[:], ACT.Sigmoid, bias=m1[:], scale=-1.0
                )
                w2 = gpool.tile([128, 1], f32, name="w2")
                nc.vector.tensor_scalar(
                    w2[:], w1[:], -1.0, 1.0, op0=OP.mult, op1=OP.add
                )
                g1 = gpool.tile([128, E_SZ], f32, name="g1")
                nc.vector.tensor_scalar(g1[:], mask1[:], w1[:], None, op0=OP.mult)
                gates = gpool.tile([128, E_SZ], f32, name=f"gates{tt}")
                nc.vector.tensor_scalar(
                    gates[:], mask2[:], w2[:], None, op0=OP.mult
                )
                nc.vector.tensor_tensor(gates[:], gates[:], g1[:], op=OP.add)
                gates_t[tt] = gates

            def base_mms(po, w_t, g, tts, tt0):
                for k8 in range(8):
                    kc = g * 8 + k8
                    for tt in tts:
                        nc.tensor.matmul(
                            po[tt - tt0][:],
                            xh(kc)[:, tt * 128 : (tt + 1) * 128],
                            w_t[:, k8, :],
                            start=(kc == 0),
                            stop=False,
                        )

            def tails(po, ot, tts, tt0):
                osl = slice(ot * 512, (ot + 1) * 512)
                for pair in range(len(tts) // 2):
                    ob = opool.tile([128, 2, 512], f32, name="ob")
                    for j in range(2):
                        tt = tts[pair * 2 + j]
                        nc.tensor.matmul(
                            po[tt - tt0][:], ghh_sb[:, tt, :], bch_sb[:, osl],
                            start=False, stop=True,
                        )
                        nc.vector.tensor_copy(ob[:, j, :], po[tt - tt0][:])
                    if ot == NO - 1:
                        oeng = (nc.gpsimd, nc.sync, nc.scalar)[pair % 3]
                    else:
                        oeng = nc.gpsimd
                    oeng.dma_start(out_d[ot, (tts[pair * 2]) // 2], ob[:])

            # ---- phase 2, ot=0: two half-passes of 4 token tiles each so
            # 4 PSUM banks stay free for the gate chain; the 4 W chunks are
            # held resident and reused by the second half
            po_a = [
                psum.tile([128, 512], f32, tag="bank", name=f"poa{i}")
                for i in range(4)
            ]
            for g in range(4):
                w_t = wpool.tile([128, 8, 512], bf16, name="w_t")
                nc.scalar.dma_start(w_t[:], wt_d[0, g])
                base_mms(po_a, w_t, g, range(4), 0)
                gate_chain(2 * g)
                gate_chain(2 * g + 1)
            # gate finalize: expand gates to (e,r) rows, ghh = gates_exp * h
            for tt in range(NT):
                ts = slice(tt * 128, (tt + 1) * 128)
                ptr = psum.tile([E_SZ, 128], f32, tag="bank", name=f"ptr{tt}")
                nc.tensor.transpose(ptr[:], gates_t[tt][:], ident_sb[:])
                gT = gpool.tile([E_SZ, 128], f32, name="gT")
                nc.vector.tensor_copy(gT[:], ptr[:])
                pge = psum.tile([128, 128], f32, tag="bank", name=f"pge{tt}")
                nc.tensor.matmul(pge[:], sel_sb[:], gT[:], start=True, stop=True)
                nc.vector.tensor_tensor(
                    ghh_sb[:, tt, :], pge[:], hT_sb[:, ts], op=OP.mult
                )
            tails(po_a, 0, [0, 1, 2, 3], 0)
            po_b = [
                psum.tile([128, 512], f32, tag="bank", name=f"pob{i}")
                for i in range(4)
            ]
            for g in range(4):
                w_t = wpool.tile([128, 8, 512], bf16, name="w_t")
                nc.scalar.dma_start(w_t[:], wt_d[0, g])
                base_mms(po_b, w_t, g, range(4, 8), 4)
            tails(po_b, 0, [4, 5, 6, 7], 4)

            # ---- phase 2, ot=1..7: streaming full-width passes ----
            for ot in range(1, NO):
                po = [
                    psum.tile([128, 512], f32, tag="bank", name=f"po{ot}_{i}")
                    for i in range(NT)
                ]
                for g in range(NK // 8):
                    w_t = wpool.tile([128, 8, 512], bf16, name="w_t")
                    eng = nc.sync if (ot * 4 + g) % 2 == 0 else nc.scalar
                    eng.dma_start(w_t[:], wt_d[ot, g])
                    base_mms(po, w_t, g, range(NT), 0)
                tails(po, ot, list(range(NT)), 0)

          if loop_n is None:
              body()
          else:
              with tc.For_i(0, loop_n, 1) as iv:
                  body(iv)

    nc.compile()
    return nc


def _split_bf16(a):
    import ml_dtypes

    hi = a.astype(ml_dtypes.bfloat16)
    lo = (a - hi.astype(np.float32)).astype(ml_dtypes.bfloat16)
    return hi, lo


def _host_prep(x, language_ids, W, Wr, A, B):
    x = np.asarray(x, dtype=np.float32)
    W = np.asarray(W, dtype=np.float32)
    Wr = np.asarray(Wr, dtype=np.float32)
    A = np.asarray(A, dtype=np.float32)
    B = np.asarray(B, dtype=np.float32)
    lang = np.asarray(language_ids).astype(np.int64)

    xf = np.ascontiguousarray(x.reshape(B_SZ * T_SZ, D_SZ))

    # W^T [D,O] bf16 hi: [ot, g8, p, k8, n]
    wtT = W.T.reshape(NK, 128, NO, 512)                   # [kc, p, ot, n]
    wh, _ = _split_bf16(wtT)
    wt = np.ascontiguousarray(
        wh.reshape(NK // 8, 8, 128, NO, 512).transpose(3, 0, 2, 1, 4)
    )

    acat_t = np.ascontiguousarray(A.reshape(ER, D_SZ).T).reshape(NK, 128, ER)
    ah, _ = _split_bf16(acat_t)
    acat = np.ascontiguousarray(ah.transpose(1, 0, 2))    # [p, kc, ER]

    wrtT = np.ascontiguousarray(Wr.T).reshape(NK, 128, E_SZ)
    wrh, wrl = _split_bf16(wrtT)
    wrt = np.ascontiguousarray(
        np.stack([wrh, wrl], axis=0).transpose(0, 2, 1, 3)  # [2, p, kc, E]
    )

    bcat32 = (SCALING * B.transpose(0, 2, 1)).reshape(ER, O_SZ)
    bh, _ = _split_bf16(bcat32)
    bcat = np.ascontiguousarray(bh)

    sel = np.zeros((E_SZ, ER), dtype=np.float32)
    sel[np.arange(ER) // R_SZ, np.arange(ER)] = 1.0

    in_maps = []
    for c in range(NCORES):
        shard = xf[c * TPC : (c + 1) * TPC]
        xr = np.ascontiguousarray(shard.T).reshape(NK, 128, TPC)
        xhh, xll = _split_bf16(xr)
        xh = np.ascontiguousarray(xhh.transpose(1, 0, 2))             # [p, kc, t]
        xl = np.ascontiguousarray(
            xll.reshape(NK // 4, 4, 128, TPC).transpose(0, 2, 1, 3)   # [g, p, 4, t]
        )
        b = int(lang[(c * TPC) // T_SZ])
        brow = np.zeros((E_SZ, 1), dtype=np.float32)
        if b >= 0:
            brow[b, 0] = LANG_BIAS
        in_maps.append(
            {
                "xh": xh,
                "xl": xl,
                "wt": wt,
                "acat": acat,
                "wrt": wrt,
                "bcat": bcat,
                "biasr": brow,
                "sel": sel,
            }
        )
    return in_maps


def kernel(x, language_ids, W, Wr, A, B):
    global LAST_RESULT
    from concourse.bass_utils import run_bass_kernel_spmd

    if "nc" not in _CACHE:
        _CACHE["nc"] = _build_bass()
    nc = _CACHE["nc"]

    in_maps = _host_prep(x, language_ids, W, Wr, A, B)
    res = run_bass_kernel_spmd(nc, in_maps, core_ids=list(range(NCORES)))
    LAST_RESULT = res
    outs = [
        r["out"].transpose(1, 3, 2, 0, 4).reshape(TPC, O_SZ) for r in res.results
    ]
    return np.concatenate(outs, axis=0).reshape(B_SZ, T_SZ, O_SZ)
